# revision 40
# baseline (speedup 1.0000x reference)
"""Trainium2 Bass kernel for MoE (nn_MoE_42975442763861).

Expert parallelism across 8 NeuronCores: core e owns expert e.

Structure:
  - distributed gate: each core computes fp32 logits for its T/8-token
    slice, AllGather (tiny) replicates them.
  - vectorized routing: every core derives, for all tokens, the top-2
    experts, combine weights, and per-expert compacted slot positions;
    from that it builds (a) its own expert's gather list (token id +
    weight per capacity slot) and (b) for its own T/8 tokens the two
    (expert, slot) addresses used by the final combine.
  - expert MLP with F-quarter loop: weights are streamed exactly once;
    fc2 partials accumulate into a bf16 SBUF accumulator across the 4
    quarters; the final pass adds b2, scales by the combine weight and
    stores the compacted [C, H] bf16 expert output.
  - combine: AllGather of the compacted outputs (E*C rows), then each
    core gathers the two contribution rows per own token and adds.
"""

import sys

for p in ("/opt/trn_rl_repo", "/root/.axon_site/_ro/trn_rl_repo"):
    if p not in sys.path:
        sys.path.insert(0, p)

import numpy as np
import ml_dtypes

import concourse.bass as bass
import concourse.bacc as bacc
import concourse.tile as tile
from concourse import mybir
from concourse.bass import IndirectOffsetOnAxis
from concourse.bass_utils import run_bass_kernel_spmd
from concourse.masks import make_identity

F32 = mybir.dt.float32
BF16 = mybir.dt.bfloat16
I32 = mybir.dt.int32
AL = mybir.AluOpType
AF = mybir.ActivationFunctionType
BF16NP = ml_dtypes.bfloat16

E = 8           # experts == cores
T = 4096        # tokens
H = 2048        # hidden
F = 8192        # intermediate
C = 1152        # per-expert token capacity (9*128); actual max count is 1076
TS = T // E     # 512 tokens owned per core
NT = T // 128   # 32 token tiles (routing layout: token t = i*128 + p)
NTO = TS // 128  # 4 own token tiles
NC9 = C // 128  # 9 capacity tiles
NHC = H // 128  # 16 H chunks of 128
NFC = F // 128  # 64 f chunks
NFQ = 4         # F quarters
LFC = NFC // NFQ  # 16 f chunks per quarter
H4 = 512        # fc2 output column block
NH4 = H // H4   # 4
CC3 = C // 3    # 384-col PSUM chunks for fc1

_CACHE = {}

def _enable_jax_cache():
    try:
        import jax
        jax.config.update("jax_compilation_cache_dir", "/tmp/moe_jax_cache")
        jax.config.update("jax_persistent_cache_min_entry_size_bytes", -1)
        jax.config.update("jax_persistent_cache_min_compile_time_secs", 0.0)
    except Exception:
        pass

_enable_jax_cache()


def _build():
    nc = bacc.Bacc("TRN2", target_bir_lowering=False, debug=False, num_devices=E)

    # ---- I/O ----
    hx = nc.dram_tensor("hx", [T, H], BF16, kind="ExternalInput").ap()       # X bf16 (gather src)
    hts = nc.dram_tensor("hts", [H, TS], F32, kind="ExternalInput").ap()     # own X^T slice fp32
    gw = nc.dram_tensor("gw", [H, E], F32, kind="ExternalInput").ap()
    gb = nc.dram_tensor("gb", [E, 1], F32, kind="ExternalInput").ap()
    w1 = nc.dram_tensor("w1", [NFC // 2, 128, 2 * NHC * 128], BF16, kind="ExternalInput").ap()  # [fcpair, p, (j, hc, f)]
    b1 = nc.dram_tensor("b1", [128, NFC], F32, kind="ExternalInput").ap()
    w2 = nc.dram_tensor("w2", [NFQ, NH4, 128, LFC * H4], BF16, kind="ExternalInput").ap()   # [fq, h4, p, (lfc, 512)]
    b2 = nc.dram_tensor("b2", [128, H], BF16, kind="ExternalInput").ap()              # broadcast
    alb = nc.dram_tensor("alb", [128, E], F32, kind="ExternalInput").ap()             # alpha bcast
    ohb = nc.dram_tensor("ohb", [128, E], F32, kind="ExternalInput").ap()             # own-expert onehot
    ecol = nc.dram_tensor("ecol", [128, E], F32, kind="ExternalInput").ap()           # e*C bcast
    sel2 = nc.dram_tensor("sel2", [64, E], F32, kind="ExternalInput").ap()            # own-col selector
    io3 = nc.dram_tensor("io3", [128, NT, 3], BF16, kind="ExternalInput").ap()        # (p, i, 1)
    srow = nc.dram_tensor("srow", [128, C], mybir.dt.float16, kind="ExternalInput").ap()  # slot iota bcast
    utri = nc.dram_tensor("utri", [128, 128], F32, kind="ExternalInput").ap()
    out = nc.dram_tensor("out", [TS, H], F32, kind="ExternalOutput").ap()

    # internal DRAM
    lg_my = nc.dram_tensor("lg_my", [TS, E], F32).ap()
    lg_all = nc.dram_tensor("lg_all", [T, E], F32, addr_space="Shared").ap()
    yexps = [nc.dram_tensor(f"yexp{k}", [C, 2 * H4], BF16).ap() for k in range(2)]
    yalls = [nc.dram_tensor(f"yall{k}", [E * C, 2 * H4], BF16, addr_space="Shared").ap()
             for k in range(2)]

    with tile.TileContext(nc) as tc:
        with (
            tc.tile_pool(name="cst", bufs=1) as cst,
            tc.tile_pool(name="keep", bufs=1) as keep,
        ):
            # ---------- long-lived constants ----------
            idf = cst.tile([128, 128], F32)
            make_identity(nc, idf[:])
            idb = cst.tile([128, 128], BF16)
            make_identity(nc, idb[:])
            b1_sb = cst.tile([128, NFC], F32)
            nc.sync.dma_start(b1_sb[:], b1)
            b2_sb = cst.tile([128, H], BF16)
            nc.sync.dma_start(b2_sb[:], b2)
            gcols = keep.tile([128, NC9], I32)
            wcols = keep.tile([128, NC9], F32)
            a1own = keep.tile([128, NTO], I32)
            a2own = keep.tile([128, NTO], I32)

            # =================== phase 1: gate + routing ===================
            with (
                tc.tile_pool(name="rt", bufs=1) as rt,
                tc.tile_pool(name="eq", bufs=2) as eqp,
                tc.tile_pool(name="pst", bufs=1, space="PSUM") as pst,
                tc.tile_pool(name="psq", bufs=3, space="PSUM") as psq,
            ):
                gw_sb = rt.tile([128, NHC * E], F32)
                nc.sync.dma_start(gw_sb[:].rearrange("p (c e) -> p c e", e=E),
                                  gw.rearrange("(c p) e -> p c e", p=128))
                gb_sb = rt.tile([E, 1], F32)
                nc.sync.dma_start(gb_sb[:], gb)
                al_sb = rt.tile([128, E], F32)
                nc.sync.dma_start(al_sb[:], alb)
                oh_sb = rt.tile([128, E], F32)
                nc.sync.dma_start(oh_sb[:], ohb)
                ec_sb = rt.tile([128, E], F32)
                nc.sync.dma_start(ec_sb[:], ecol)
                sel_sb = rt.tile([64, E], F32)
                nc.sync.dma_start(sel_sb[:], sel2)
                io3_sb = rt.tile([128, NT * 3], BF16)
                nc.sync.dma_start(io3_sb[:].rearrange("p (a b) -> p a b", b=3), io3)
                srow_sb = rt.tile([128, C], mybir.dt.float16)
                nc.sync.dma_start(srow_sb[:], srow)
                utri_sb = rt.tile([128, 128], F32)
                nc.sync.dma_start(utri_sb[:], utri)

                # ---- local gate on own token slice: logitsT [E, TS] ----
                hts_sb = rt.tile([128, NHC * TS], F32)
                nc.sync.dma_start(hts_sb[:].rearrange("p (c t) -> p c t", t=TS),
                                  hts.rearrange("(c p) t -> p c t", p=128))
                with tc.tile_pool(name="psg", bufs=1, space="PSUM") as psg:
                    pg = psg.tile([E, TS], F32, space="PSUM", tag="pg")
                    for hc in range(NHC):
                        nc.tensor.matmul(pg[:], gw_sb[:, hc * E:(hc + 1) * E],
                                         hts_sb[:, hc * TS:(hc + 1) * TS],
                                         start=(hc == 0), stop=(hc == NHC - 1))
                    lgT = rt.tile([E, TS], F32)
                    nc.vector.tensor_scalar_add(lgT[:], pg[:], gb_sb[:, :1])

                    # transpose to token-major [128, NTO, 8] and ship to DRAM
                    ptg = psg.tile([128, NTO * E], F32, space="PSUM", tag="ptg")
                    for i in range(NTO):
                        nc.tensor.transpose(ptg[:, i * E:(i + 1) * E],
                                            lgT[:, i * 128:(i + 1) * 128], idf[:E, :E])
                    lgtok = rt.tile([128, NTO * E], F32)
                    nc.vector.tensor_copy(out=lgtok[:], in_=ptg[:])
                    nc.sync.dma_start(lg_my.rearrange("(i p) e -> p i e", p=128),
                                      lgtok[:].rearrange("p (i e) -> p i e", e=E))

                # ---- AllGather logits ----
                nc.gpsimd.collective_compute(
                    "AllGather", AL.bypass,
                    replica_groups=[list(range(E))],
                    ins=[lg_my.opt()], outs=[lg_all.opt()])

                ltok = rt.tile([128, NT * E], F32)
                nc.sync.dma_start(ltok[:].rearrange("p (i e) -> p i e", e=E),
                                  lg_all.rearrange("(i p) e -> p i e", p=128))

                # ---- routing math, vectorized over all E experts ----
                mx = rt.tile([128, NT * E], F32)
                for i in range(NT):
                    nc.vector.max(mx[:, i * E:(i + 1) * E], ltok[:, i * E:(i + 1) * E])
                lt3 = ltok[:].rearrange("p (i e) -> p i e", e=E)
                mx3 = mx[:].rearrange("p (i e) -> p i e", e=E)
                m1 = mx3[:, :, 0:1]
                m2 = mx3[:, :, 1:2]

                d12 = rt.tile([128, NT], F32)
                nc.vector.tensor_tensor(
                    out=d12[:].rearrange("p (i o) -> p i o", o=1),
                    in0=m1, in1=m2, op=AL.subtract)
                s1 = rt.tile([128, NT], F32)
                nc.scalar.activation(s1[:], d12[:], AF.Sigmoid)
                s2 = rt.tile([128, NT], F32)
                nc.scalar.activation(s2[:], d12[:], AF.Sigmoid, scale=-1.0)

                # EQ1/EQ2: which expert is top-1 / top-2  [128, NT, E]
                eq1 = rt.tile([128, NT * E], F32)
                e13 = eq1[:].rearrange("p (i e) -> p i e", e=E)
                nc.vector.tensor_tensor(out=e13, in0=lt3,
                                        in1=m1.to_broadcast([128, NT, E]),
                                        op=AL.is_equal)
                eq2 = rt.tile([128, NT * E], F32)
                e23 = eq2[:].rearrange("p (i e) -> p i e", e=E)
                nc.vector.tensor_tensor(out=e23, in0=lt3,
                                        in1=m2.to_broadcast([128, NT, E]),
                                        op=AL.is_equal)

                # combine weights W = (s1*EQ1 + s2*EQ2) * alpha  [128, NT, E]
                wge = rt.tile([128, NT * E], F32)
                w3 = wge[:].rearrange("p (i e) -> p i e", e=E)
                t1e = rt.tile([128, NT * E], F32)
                t13 = t1e[:].rearrange("p (i e) -> p i e", e=E)
                s13 = s1[:].rearrange("p (i o) -> p i o", o=1)
                s23 = s2[:].rearrange("p (i o) -> p i o", o=1)
                nc.vector.tensor_tensor(out=t13, in0=e13,
                                        in1=s13.to_broadcast([128, NT, E]), op=AL.mult)
                nc.vector.tensor_tensor(out=w3, in0=e23,
                                        in1=s23.to_broadcast([128, NT, E]), op=AL.mult)
                nc.vector.tensor_add(wge[:], wge[:], t1e[:])
                al3 = al_sb[:].rearrange("p (o e) -> p o e", o=1)
                nc.vector.tensor_tensor(out=w3, in0=w3,
                                        in1=al3.to_broadcast([128, NT, E]), op=AL.mult)

                # msk = EQ1 + EQ2
                mske = rt.tile([128, NT * E], F32)
                nc.vector.tensor_add(mske[:], eq1[:], eq2[:])
                mk3 = mske[:].rearrange("p (i e) -> p i e", e=E)

                # inclusive cumsum along i (token-tile axis), all experts at once
                cumA = rt.tile([128, NT * E], F32)
                cumB = rt.tile([128, NT * E], F32)
                nc.vector.tensor_copy(out=cumA[:], in_=mske[:])
                src, dst = cumA, cumB
                for s in (1, 2, 4, 8, 16):
                    s3 = src[:].rearrange("p (i e) -> p i e", e=E)
                    d3 = dst[:].rearrange("p (i e) -> p i e", e=E)
                    nc.scalar.copy(out=d3[:, :s, :], in_=s3[:, :s, :])
                    nc.vector.tensor_tensor(out=d3[:, s:, :], in0=s3[:, s:, :],
                                            in1=s3[:, :NT - s, :], op=AL.add)
                    src, dst = dst, src
                incl = src
                ic3 = incl[:].rearrange("p (i e) -> p i e", e=E)

                rowtot = rt.tile([128, E], F32)
                nc.vector.tensor_copy(
                    out=rowtot[:].rearrange("p (o e) -> p o e", o=1),
                    in_=ic3[:, NT - 1:NT, :])
                pro = pst.tile([128, E], F32, space="PSUM", tag="pro")
                nc.tensor.matmul(pro[:], utri_sb[:], rowtot[:], start=True, stop=True)
                rowoff = rt.tile([128, E], F32)
                nc.vector.tensor_copy(out=rowoff[:], in_=pro[:])

                # slot_e(t) = rowoff_e + incl_e - msk_e   [128, NT, E]
                slte = rt.tile([128, NT * E], F32)
                sl3 = slte[:].rearrange("p (i e) -> p i e", e=E)
                nc.vector.tensor_sub(slte[:], incl[:], mske[:])
                ro3 = rowoff[:].rearrange("p (o e) -> p o e", o=1)
                nc.vector.tensor_tensor(out=sl3, in0=sl3,
                                        in1=ro3.to_broadcast([128, NT, E]), op=AL.add)

                # ---- own-expert selection for compaction ----
                msk = rt.tile([128, NT], F32)
                oh3 = oh_sb[:].rearrange("p (o e) -> p o e", o=1)
                sel_tmp = rt.tile([128, NT * E], F32)
                st3 = sel_tmp[:].rearrange("p (i e) -> p i e", e=E)
                nc.vector.tensor_tensor(out=st3, in0=mk3,
                                        in1=oh3.to_broadcast([128, NT, E]), op=AL.mult)
                nc.vector.tensor_reduce(out=msk[:], in_=st3,
                                        axis=mybir.AxisListType.X, op=AL.add)
                slot = rt.tile([128, NT], F32)
                nc.vector.tensor_tensor(out=st3, in0=sl3,
                                        in1=oh3.to_broadcast([128, NT, E]), op=AL.mult)
                nc.vector.tensor_reduce(out=slot[:], in_=st3,
                                        axis=mybir.AxisListType.X, op=AL.add)
                wgt = rt.tile([128, NT], F32)
                nc.vector.tensor_tensor(out=st3, in0=w3,
                                        in1=oh3.to_broadcast([128, NT, E]), op=AL.mult)
                nc.vector.tensor_reduce(out=wgt[:], in_=st3,
                                        axis=mybir.AxisListType.X, op=AL.add)

                # masked-out -> +1e6 so compaction misses every slot column
                nc.vector.scalar_tensor_tensor(
                    out=slot[:], in0=msk[:], scalar=-1e6, in1=slot[:],
                    op0=AL.mult, op1=AL.add)
                nc.vector.tensor_scalar_add(slot[:], slot[:], 1e6)

                # w split into exact bf16 hi/lo for the compaction matmul
                whi = rt.tile([128, NT], BF16)
                nc.vector.tensor_copy(out=whi[:], in_=wgt[:])
                whi32 = rt.tile([128, NT], F32)
                nc.vector.tensor_copy(out=whi32[:], in_=whi[:])
                wlo32 = rt.tile([128, NT], F32)
                nc.vector.tensor_sub(wlo32[:], wgt[:], whi32[:])

                # lhs5[p, i, :] = [p, i, 1, w_hi, w_lo]  (bf16)
                lhs5 = rt.tile([128, NT * 5], BF16)
                l53 = lhs5[:].rearrange("p (i c) -> p i c", c=5)
                nc.vector.tensor_copy(out=l53[:, :, 0:3],
                                      in_=io3_sb[:].rearrange("p (i c) -> p i c", c=3))
                nc.vector.tensor_copy(out=l53[:, :, 3:4],
                                      in_=whi32[:].rearrange("p (i o) -> p i o", o=1))
                nc.vector.tensor_copy(out=l53[:, :, 4:5],
                                      in_=wlo32[:].rearrange("p (i o) -> p i o", o=1))

                # compaction matmuls: rows = [sum p*EQ, sum i*EQ, colsum, w_hi, w_lo]
                slot16 = rt.tile([128, NT], mybir.dt.float16)
                nc.vector.tensor_copy(out=slot16[:], in_=slot[:])
                ccs = [(0, 512), (512, 512), (1024, C - 1024)]
                pqs = []
                for (_, n) in ccs:
                    pq_t = psq.tile([5, n], F32, space="PSUM", tag="pq")
                    pqs.append(pq_t)
                for i in range(NT):
                    eq = eqp.tile([128, C], BF16, tag="eqt")
                    nc.vector.tensor_tensor(
                        out=eq[:], in0=slot16[:, i:i + 1].to_broadcast([128, C]),
                        in1=srow_sb[:], op=AL.is_equal)
                    for ci, (c0, n) in enumerate(ccs):
                        nc.tensor.matmul(pqs[ci][:], lhs5[:, i * 5:(i + 1) * 5],
                                         eq[:, c0:c0 + n],
                                         start=(i == 0), stop=(i == NT - 1))

                # transpose [5, C] -> per-slot columns [128, NC9, 5]
                qs = rt.tile([5, C], F32)
                for ci, (c0, n) in enumerate(ccs):
                    nc.scalar.copy(out=qs[:, c0:c0 + n], in_=pqs[ci][:])
                ptc = pst.tile([128, NC9 * 5], F32, space="PSUM", tag="ptc")
                for t9 in range(NC9):
                    nc.tensor.transpose(ptc[:, t9 * 5:(t9 + 1) * 5],
                                        qs[:, t9 * 128:(t9 + 1) * 128], idf[:5, :5])
                qcols = rt.tile([128, NC9 * 5], F32)
                nc.scalar.copy(out=qcols[:], in_=ptc[:])
                q3 = qcols[:].rearrange("p (t c) -> p t c", c=5)

                gi_f = rt.tile([128, NC9], F32)
                g3 = gi_f[:].rearrange("p (t o) -> p t o", o=1)
                nc.vector.scalar_tensor_tensor(
                    out=g3, in0=q3[:, :, 1:2], scalar=128.0,
                    in1=q3[:, :, 0:1], op0=AL.mult, op1=AL.add)
                nc.vector.scalar_tensor_tensor(
                    out=g3, in0=q3[:, :, 2:3], scalar=-1e6,
                    in1=g3, op0=AL.mult, op1=AL.add)
                nc.vector.tensor_scalar_add(gi_f[:], gi_f[:], 1e6)
                nc.vector.tensor_copy(out=gcols[:], in_=gi_f[:])
                nc.vector.tensor_tensor(
                    out=wcols[:].rearrange("p (t o) -> p t o", o=1),
                    in0=q3[:, :, 3:4], in1=q3[:, :, 4:5], op=AL.add)

                # combine addresses: addr_e = slot_e + e*C; A1/A2 select top-1/2
                adre = rt.tile([128, NT * E], F32)
                ad3 = adre[:].rearrange("p (i e) -> p i e", e=E)
                ec3 = ec_sb[:].rearrange("p (o e) -> p o e", o=1)
                nc.vector.tensor_tensor(out=ad3, in0=sl3,
                                        in1=ec3.to_broadcast([128, NT, E]), op=AL.add)
                a12 = rt.tile([128, 2 * NT], F32)
                tmp3 = t1e  # reuse as scratch
                tm3 = tmp3[:].rearrange("p (i e) -> p i e", e=E)
                nc.vector.tensor_tensor(out=tm3, in0=ad3, in1=e13, op=AL.mult)
                nc.vector.tensor_reduce(
                    out=a12[:, :NT], in_=tm3,
                    axis=mybir.AxisListType.X, op=AL.add)
                nc.vector.tensor_tensor(out=tm3, in0=ad3, in1=e23, op=AL.mult)
                nc.vector.tensor_reduce(
                    out=a12[:, NT:], in_=tm3,
                    axis=mybir.AxisListType.X, op=AL.add)

                # select own-token columns: ownT = sel2.T @ (A12.T)  -> [8, 128]
                with tc.tile_pool(name="psel", bufs=1, space="PSUM") as psel:
                    pt12 = psel.tile([64, 128], F32, space="PSUM", tag="pt12")
                    nc.tensor.transpose(pt12[:], a12[:], idf[:])
                    a12T = rt.tile([64, 128], F32)
                    nc.vector.tensor_copy(out=a12T[:], in_=pt12[:])
                    pown = psel.tile([E, 128], F32, space="PSUM", tag="pown")
                    nc.tensor.matmul(pown[:], sel_sb[:], a12T[:], start=True, stop=True)
                    ownT = rt.tile([E, 128], F32)
                    nc.vector.tensor_copy(out=ownT[:], in_=pown[:])
                    pow2 = psel.tile([128, E], F32, space="PSUM", tag="pow2")
                    nc.tensor.transpose(pow2[:], ownT[:], idf[:E, :E])
                    ownf = rt.tile([128, E], F32)
                    nc.vector.tensor_copy(out=ownf[:], in_=pow2[:])
                    nc.vector.tensor_copy(out=a1own[:], in_=ownf[:, :NTO])
                    nc.vector.tensor_copy(out=a2own[:], in_=ownf[:, NTO:])

            # =================== phase 2: gather + expert MLP ===================
            with (
                tc.tile_pool(name="xth", bufs=1) as xthp,
                tc.tile_pool(name="hh", bufs=1) as hhp,
                tc.tile_pool(name="yac", bufs=1) as yacp,
                tc.tile_pool(name="w1p", bufs=2) as w1p,
                tc.tile_pool(name="w2p", bufs=3) as w2p,
                tc.tile_pool(name="stg", bufs=2) as stg,
                tc.tile_pool(name="ocv", bufs=2) as ocv,
            ):
                xth = xthp.tile([128, NHC * C], BF16)
                xt3 = xth[:].rearrange("p (h c) -> p h c", c=C)
                hh = hhp.tile([128, LFC * C], BF16)
                hh3 = hh[:].rearrange("p (f c) -> p f c", c=C)
                yacc = yacp.tile([128, NC9 * H], BF16)
                ya3 = yacc[:].rearrange("p (t h) -> p t h", h=H)

                # gather + transpose X^T for all capacity slots: [128, NHC, C].
                # One xbar DMA transpose per token tile: [128 tok, 2048 h] ->
                # [128, 16 hc, 128 tok] with h = hc*128 + p.
                with tc.tile_pool(name="xgp", bufs=2) as xgp:
                    for t9 in range(NC9):
                        xg = xgp.tile([128, H], BF16, tag="xg")
                        nc.gpsimd.indirect_dma_start(
                            out=xg[:], out_offset=None, in_=hx[:, :],
                            in_offset=IndirectOffsetOnAxis(
                                ap=gcols[:, t9:t9 + 1], axis=0),
                            bounds_check=T - 1, oob_is_err=False)
                        nc.sync.dma_start_transpose(
                            out=xt3[:, :, t9 * 128:(t9 + 1) * 128], in_=xg[:])

                with (
                    tc.tile_pool(name="psf", bufs=2, space="PSUM") as psf,
                    tc.tile_pool(name="psy", bufs=2, space="PSUM") as psy,
                ):
                  for fq in range(NFQ):
                    # fc1 quarter: hh = gelu(W1q.T @ X^T + b1q), full C.
                    # hc is the outer loop so each weight tile stays stationary
                    # for the 3 token chunks (one LDWEIGHTS per 3 matmuls).
                    for fcp in range(LFC // 2):
                        w1t = w1p.tile([128, 2 * NHC * 128], BF16, tag="w1t")
                        nc.sync.dma_start(w1t[:], w1[fq * (LFC // 2) + fcp, :, :])
                        for j in range(2):
                            lfc = fcp * 2 + j
                            fc = fq * LFC + lfc
                            pfs = [psf.tile([128, CC3], F32, space="PSUM",
                                            tag=f"pf{k}", name=f"pf{k}")
                                   for k in range(3)]
                            for hc in range(NHC):
                                for tc3 in range(3):
                                    nc.tensor.matmul(
                                        pfs[tc3][:],
                                        w1t[:, (j * NHC + hc) * 128:(j * NHC + hc + 1) * 128],
                                        xt3[:, hc, tc3 * CC3:(tc3 + 1) * CC3],
                                        start=(hc == 0), stop=(hc == NHC - 1))
                            for tc3 in range(3):
                                nc.scalar.activation(
                                    hh3[:, lfc, tc3 * CC3:(tc3 + 1) * CC3], pfs[tc3][:],
                                    AF.Gelu, bias=b1_sb[:, fc:fc + 1])

                    # fc2 quarter: accumulate into yacc
                    for h4 in range(NH4):
                        w2t = w2p.tile([128, LFC * H4], BF16, tag="w2t")
                        nc.sync.dma_start(w2t[:], w2[fq, h4, :, :])
                        w23 = w2t[:].rearrange("p (f h) -> p f h", h=H4)
                        for tt in range(NC9):
                            py = psy.tile([128, H4], F32, space="PSUM", tag="py")
                            for lfc in range(LFC):
                                nc.tensor.matmul(
                                    py[:], hh3[:, lfc, tt * 128:(tt + 1) * 128],
                                    w23[:, lfc, :],
                                    start=(lfc == 0), stop=(lfc == LFC - 1))
                            yv = ya3[:, tt, h4 * H4:(h4 + 1) * H4]
                            if fq == 0:
                                nc.vector.tensor_add(yv, py[:],
                                                     b2_sb[:, h4 * H4:(h4 + 1) * H4])
                            elif fq < NFQ - 1:
                                nc.vector.tensor_add(yv, yv, py[:])
                            else:
                                yfin = stg.tile([128, H4], F32, tag="yfin")
                                nc.vector.tensor_add(yfin[:], yv, py[:])
                                ystb = stg.tile([128, H4], BF16, tag="ystb")
                                nc.vector.tensor_scalar(
                                    out=ystb[:], in0=yfin[:],
                                    scalar1=wcols[:, tt:tt + 1],
                                    scalar2=None, op0=AL.mult)
                                nc.sync.dma_start(
                                    yexps[h4 // 2][tt * 128:(tt + 1) * 128,
                                                   (h4 % 2) * H4:(h4 % 2 + 1) * H4],
                                    ystb[:])
                        if fq == NFQ - 1 and h4 == 1:
                            # first column half is complete: AllGather it now so
                            # the collective overlaps the remaining fc2 work.
                            nc.gpsimd.collective_compute(
                                "AllGather", AL.bypass,
                                replica_groups=[list(range(E))],
                                ins=[yexps[0].opt()], outs=[yalls[0].opt()])

                # ---- combine: gather the two contribution rows per own token.
                # The first half's gathers are queued before the second AG so
                # they run while the second half's collective is in flight.
                for hp in range(2):
                    for tt in range(NTO):
                        ya = ocv.tile([128, 2 * H4], BF16, tag="ya")
                        nc.gpsimd.indirect_dma_start(
                            out=ya[:], out_offset=None, in_=yalls[hp][:, :],
                            in_offset=IndirectOffsetOnAxis(
                                ap=a1own[:, tt:tt + 1], axis=0),
                            bounds_check=E * C - 1, oob_is_err=False)
                        yb = ocv.tile([128, 2 * H4], BF16, tag="yb")
                        nc.gpsimd.indirect_dma_start(
                            out=yb[:], out_offset=None, in_=yalls[hp][:, :],
                            in_offset=IndirectOffsetOnAxis(
                                ap=a2own[:, tt:tt + 1], axis=0),
                            bounds_check=E * C - 1, oob_is_err=False)
                        yo = ocv.tile([128, 2 * H4], F32, tag="yo")
                        nc.vector.tensor_add(yo[:], ya[:], yb[:])
                        nc.sync.dma_start(
                            out[tt * 128:(tt + 1) * 128,
                                hp * 2 * H4:(hp + 1) * 2 * H4],
                            yo[:])
                    if hp == 0:
                        nc.gpsimd.collective_compute(
                            "AllGather", AL.bypass,
                            replica_groups=[list(range(E))],
                            ins=[yexps[1].opt()], outs=[yalls[1].opt()])

    nc.compile()
    return nc


def _host_prep(inputs):
    x = np.ascontiguousarray(inputs["hidden_states"].reshape(T, H).astype(np.float32))
    ht = np.ascontiguousarray(x.T)
    hx = x.astype(BF16NP)
    gw = np.ascontiguousarray(inputs["gate_w"].astype(np.float32))
    gb = np.ascontiguousarray(inputs["gate_b"].astype(np.float32).reshape(E, 1))
    srow = np.ascontiguousarray(
        np.broadcast_to(np.arange(C, dtype=np.float16), (128, C)))
    utri = np.triu(np.ones((128, 128), np.float32), k=1)
    io3 = np.empty((128, NT, 3), BF16NP)
    io3[:, :, 0] = np.arange(128, dtype=np.float32)[:, None]
    io3[:, :, 1] = np.arange(NT, dtype=np.float32)[None, :]
    io3[:, :, 2] = 1.0
    alb = np.ascontiguousarray(
        np.broadcast_to(inputs["alpha"].astype(np.float32), (128, E)))
    ecol = np.ascontiguousarray(
        np.broadcast_to((np.arange(E) * C).astype(np.float32), (128, E)))

    maps = []
    for e in range(E):
        w1e = inputs["fc1_w"][e].astype(BF16NP)          # [H, F]
        w1p = np.ascontiguousarray(
            w1e.reshape(NHC, 128, NFC // 2, 2, 128).transpose(2, 1, 3, 0, 4)
        ).reshape(NFC // 2, 128, 2 * NHC * 128)
        w2e = inputs["fc2_w"][e].astype(BF16NP)          # [F, H]
        w2p = np.ascontiguousarray(
            w2e.reshape(NFQ, LFC, 128, NH4, H4).transpose(0, 3, 2, 1, 4)
        ).reshape(NFQ, NH4, 128, LFC * H4)
        b1e = np.ascontiguousarray(
            inputs["fc1_b"][e].astype(np.float32).reshape(NFC, 128).T)
        b2e = np.ascontiguousarray(
            np.broadcast_to(inputs["fc2_b"][e].astype(BF16NP), (128, H)))
        ohe = np.zeros((128, E), np.float32)
        ohe[:, e] = 1.0
        hts = np.ascontiguousarray(ht[:, e * TS:(e + 1) * TS])
        sel = np.zeros((64, E), np.float32)
        for j in range(NTO):
            sel[e * NTO + j, j] = 1.0
            sel[NT + e * NTO + j, NTO + j] = 1.0
        maps.append({
            "hx": hx, "hts": hts, "gw": gw, "gb": gb,
            "w1": w1p, "b1": b1e, "w2": w2p, "b2": b2e,
            "alb": alb, "ohb": ohe, "ecol": ecol, "sel2": sel,
            "io3": io3, "srow": srow, "utri": utri,
        })
    return maps


def kernel(**inputs):
    import os
    if "nc" not in _CACHE:
        _CACHE["nc"] = _build()
    nc = _CACHE["nc"]
    maps = _host_prep(inputs)
    kw = {}
    if os.environ.get("MOE_TRACE"):
        kw["trace"] = True
        td = os.environ.get("MOE_TRACE_DIR")
        if td:
            os.makedirs(td, exist_ok=True)
            kw["tmpdir"] = td
    bres = run_bass_kernel_spmd(nc, maps, list(range(E)), **kw)
    _CACHE["last"] = bres
    res = bres.results
    outp = np.concatenate([res[e]["out"] for e in range(E)], axis=0)
    return outp.reshape(inputs["hidden_states"].shape).astype(np.float32)


if __name__ == "__main__":
    data = np.load("/root/problem/work/inputs.npz")
    out = kernel(**{k: data[k] for k in data.files})
    print("kernel output:", out.shape, out.dtype)


# revision 41
# speedup vs baseline: 1.0065x; 1.0065x over previous
"""Trainium2 Bass kernel for MoE (nn_MoE_42975442763861).

Expert parallelism across 8 NeuronCores: core e owns expert e.

Structure:
  - distributed gate: each core computes fp32 logits for its T/8-token
    slice, AllGather (tiny) replicates them.
  - vectorized routing: every core derives, for all tokens, the top-2
    experts, combine weights, and per-expert compacted slot positions;
    from that it builds (a) its own expert's gather list (token id +
    weight per capacity slot) and (b) for its own T/8 tokens the two
    (expert, slot) addresses used by the final combine.
  - expert MLP with F-quarter loop: weights are streamed exactly once;
    fc2 partials accumulate into a bf16 SBUF accumulator across the 4
    quarters; the final pass adds b2, scales by the combine weight and
    stores the compacted [C, H] bf16 expert output.
  - combine: AllGather of the compacted outputs (E*C rows), then each
    core gathers the two contribution rows per own token and adds.
"""

import sys

for p in ("/opt/trn_rl_repo", "/root/.axon_site/_ro/trn_rl_repo"):
    if p not in sys.path:
        sys.path.insert(0, p)

import numpy as np
import ml_dtypes

import concourse.bass as bass
import concourse.bacc as bacc
import concourse.tile as tile
from concourse import mybir
from concourse.bass import IndirectOffsetOnAxis
from concourse.bass_utils import run_bass_kernel_spmd
from concourse.masks import make_identity

F32 = mybir.dt.float32
BF16 = mybir.dt.bfloat16
I32 = mybir.dt.int32
AL = mybir.AluOpType
AF = mybir.ActivationFunctionType
BF16NP = ml_dtypes.bfloat16

E = 8           # experts == cores
T = 4096        # tokens
H = 2048        # hidden
F = 8192        # intermediate
C = 1152        # per-expert token capacity (9*128); actual max count is 1076
TS = T // E     # 512 tokens owned per core
NT = T // 128   # 32 token tiles (routing layout: token t = i*128 + p)
NTO = TS // 128  # 4 own token tiles
NC9 = C // 128  # 9 capacity tiles
NHC = H // 128  # 16 H chunks of 128
NFC = F // 128  # 64 f chunks
NFQ = 4         # F quarters
LFC = NFC // NFQ  # 16 f chunks per quarter
H4 = 512        # fc2 output column block
NH4 = H // H4   # 4
CC3 = C // 3    # 384-col PSUM chunks for fc1

_CACHE = {}

def _enable_jax_cache():
    try:
        import jax
        jax.config.update("jax_compilation_cache_dir", "/tmp/moe_jax_cache")
        jax.config.update("jax_persistent_cache_min_entry_size_bytes", -1)
        jax.config.update("jax_persistent_cache_min_compile_time_secs", 0.0)
    except Exception:
        pass

_enable_jax_cache()


def _build():
    nc = bacc.Bacc("TRN2", target_bir_lowering=False, debug=False, num_devices=E)

    # ---- I/O ----
    hx = nc.dram_tensor("hx", [T, H], BF16, kind="ExternalInput").ap()       # X bf16 (gather src)
    hts = nc.dram_tensor("hts", [H, TS], F32, kind="ExternalInput").ap()     # own X^T slice fp32
    gw = nc.dram_tensor("gw", [H, E], F32, kind="ExternalInput").ap()
    gb = nc.dram_tensor("gb", [E, 1], F32, kind="ExternalInput").ap()
    w1 = nc.dram_tensor("w1", [NFC // 2, 128, 2 * NHC * 128], BF16, kind="ExternalInput").ap()  # [fcpair, p, (j, hc, f)]
    b1 = nc.dram_tensor("b1", [128, NFC], F32, kind="ExternalInput").ap()
    w2 = nc.dram_tensor("w2", [NFQ, NH4, 128, LFC * H4], BF16, kind="ExternalInput").ap()   # [fq, h4, p, (lfc, 512)]
    b2 = nc.dram_tensor("b2", [128, H], BF16, kind="ExternalInput").ap()              # broadcast
    alb = nc.dram_tensor("alb", [128, E], F32, kind="ExternalInput").ap()             # alpha bcast
    ohb = nc.dram_tensor("ohb", [128, E], F32, kind="ExternalInput").ap()             # own-expert onehot
    ecol = nc.dram_tensor("ecol", [128, E], F32, kind="ExternalInput").ap()           # e*C bcast
    sel2 = nc.dram_tensor("sel2", [64, E], F32, kind="ExternalInput").ap()            # own-col selector
    io3 = nc.dram_tensor("io3", [128, NT, 3], BF16, kind="ExternalInput").ap()        # (p, i, 1)
    srow = nc.dram_tensor("srow", [128, C], mybir.dt.float16, kind="ExternalInput").ap()  # slot iota bcast
    utri = nc.dram_tensor("utri", [128, 128], F32, kind="ExternalInput").ap()
    out = nc.dram_tensor("out", [TS, H], F32, kind="ExternalOutput").ap()

    # internal DRAM
    lg_my = nc.dram_tensor("lg_my", [TS, E], F32).ap()
    lg_all = nc.dram_tensor("lg_all", [T, E], F32, addr_space="Shared").ap()
    yexps = [nc.dram_tensor(f"yexp{k}", [C, 2 * H4], BF16).ap() for k in range(2)]
    yalls = [nc.dram_tensor(f"yall{k}", [E * C, 2 * H4], BF16, addr_space="Shared").ap()
             for k in range(2)]

    with tile.TileContext(nc) as tc:
        with (
            tc.tile_pool(name="cst", bufs=1) as cst,
            tc.tile_pool(name="keep", bufs=1) as keep,
        ):
            # ---------- long-lived constants ----------
            idf = cst.tile([128, 128], F32)
            make_identity(nc, idf[:])
            idb = cst.tile([128, 128], BF16)
            make_identity(nc, idb[:])
            b1_sb = cst.tile([128, NFC], F32)
            nc.sync.dma_start(b1_sb[:], b1)
            b2_sb = cst.tile([128, H], BF16)
            nc.sync.dma_start(b2_sb[:], b2)
            gcols = keep.tile([128, NC9], I32)
            wcols = keep.tile([128, NC9], F32)
            a1own = keep.tile([128, NTO], I32)
            a2own = keep.tile([128, NTO], I32)

            # =================== phase 1: gate + routing ===================
            with (
                tc.tile_pool(name="rt", bufs=1) as rt,
                tc.tile_pool(name="eq", bufs=2) as eqp,
                tc.tile_pool(name="pst", bufs=1, space="PSUM") as pst,
                tc.tile_pool(name="psq", bufs=3, space="PSUM") as psq,
            ):
                gw_sb = rt.tile([128, NHC * E], F32)
                nc.sync.dma_start(gw_sb[:].rearrange("p (c e) -> p c e", e=E),
                                  gw.rearrange("(c p) e -> p c e", p=128))
                gb_sb = rt.tile([E, 1], F32)
                nc.sync.dma_start(gb_sb[:], gb)
                al_sb = rt.tile([128, E], F32)
                nc.sync.dma_start(al_sb[:], alb)
                oh_sb = rt.tile([128, E], F32)
                nc.sync.dma_start(oh_sb[:], ohb)
                ec_sb = rt.tile([128, E], F32)
                nc.sync.dma_start(ec_sb[:], ecol)
                sel_sb = rt.tile([64, E], F32)
                nc.sync.dma_start(sel_sb[:], sel2)
                io3_sb = rt.tile([128, NT * 3], BF16)
                nc.sync.dma_start(io3_sb[:].rearrange("p (a b) -> p a b", b=3), io3)
                srow_sb = rt.tile([128, C], mybir.dt.float16)
                nc.sync.dma_start(srow_sb[:], srow)
                utri_sb = rt.tile([128, 128], F32)
                nc.sync.dma_start(utri_sb[:], utri)

                # ---- local gate on own token slice: logitsT [E, TS] ----
                hts_sb = rt.tile([128, NHC * TS], F32)
                nc.sync.dma_start(hts_sb[:].rearrange("p (c t) -> p c t", t=TS),
                                  hts.rearrange("(c p) t -> p c t", p=128))
                with tc.tile_pool(name="psg", bufs=1, space="PSUM") as psg:
                    pg = psg.tile([E, TS], F32, space="PSUM", tag="pg")
                    for hc in range(NHC):
                        nc.tensor.matmul(pg[:], gw_sb[:, hc * E:(hc + 1) * E],
                                         hts_sb[:, hc * TS:(hc + 1) * TS],
                                         start=(hc == 0), stop=(hc == NHC - 1))
                    lgT = rt.tile([E, TS], F32)
                    nc.vector.tensor_scalar_add(lgT[:], pg[:], gb_sb[:, :1])

                    # transpose to token-major [128, NTO, 8] and ship to DRAM
                    ptg = psg.tile([128, NTO * E], F32, space="PSUM", tag="ptg")
                    for i in range(NTO):
                        nc.tensor.transpose(ptg[:, i * E:(i + 1) * E],
                                            lgT[:, i * 128:(i + 1) * 128], idf[:E, :E])
                    lgtok = rt.tile([128, NTO * E], F32)
                    nc.vector.tensor_copy(out=lgtok[:], in_=ptg[:])
                    nc.sync.dma_start(lg_my.rearrange("(i p) e -> p i e", p=128),
                                      lgtok[:].rearrange("p (i e) -> p i e", e=E))

                # ---- AllGather logits ----
                nc.gpsimd.collective_compute(
                    "AllGather", AL.bypass,
                    replica_groups=[list(range(E))],
                    ins=[lg_my.opt()], outs=[lg_all.opt()])

                ltok = rt.tile([128, NT * E], F32)
                nc.sync.dma_start(ltok[:].rearrange("p (i e) -> p i e", e=E),
                                  lg_all.rearrange("(i p) e -> p i e", p=128))

                # ---- routing math, vectorized over all E experts ----
                mx = rt.tile([128, NT * E], F32)
                for i in range(NT):
                    nc.vector.max(mx[:, i * E:(i + 1) * E], ltok[:, i * E:(i + 1) * E])
                lt3 = ltok[:].rearrange("p (i e) -> p i e", e=E)
                mx3 = mx[:].rearrange("p (i e) -> p i e", e=E)
                m1 = mx3[:, :, 0:1]
                m2 = mx3[:, :, 1:2]

                d12 = rt.tile([128, NT], F32)
                nc.vector.tensor_tensor(
                    out=d12[:].rearrange("p (i o) -> p i o", o=1),
                    in0=m1, in1=m2, op=AL.subtract)
                s1 = rt.tile([128, NT], F32)
                nc.scalar.activation(s1[:], d12[:], AF.Sigmoid)
                s2 = rt.tile([128, NT], F32)
                nc.scalar.activation(s2[:], d12[:], AF.Sigmoid, scale=-1.0)

                # EQ1/EQ2: which expert is top-1 / top-2  [128, NT, E]
                eq1 = rt.tile([128, NT * E], F32)
                e13 = eq1[:].rearrange("p (i e) -> p i e", e=E)
                nc.vector.tensor_tensor(out=e13, in0=lt3,
                                        in1=m1.to_broadcast([128, NT, E]),
                                        op=AL.is_equal)
                eq2 = rt.tile([128, NT * E], F32)
                e23 = eq2[:].rearrange("p (i e) -> p i e", e=E)
                nc.vector.tensor_tensor(out=e23, in0=lt3,
                                        in1=m2.to_broadcast([128, NT, E]),
                                        op=AL.is_equal)

                # combine weights W = (s1*EQ1 + s2*EQ2) * alpha  [128, NT, E]
                wge = rt.tile([128, NT * E], F32)
                w3 = wge[:].rearrange("p (i e) -> p i e", e=E)
                t1e = rt.tile([128, NT * E], F32)
                t13 = t1e[:].rearrange("p (i e) -> p i e", e=E)
                s13 = s1[:].rearrange("p (i o) -> p i o", o=1)
                s23 = s2[:].rearrange("p (i o) -> p i o", o=1)
                nc.vector.tensor_tensor(out=t13, in0=e13,
                                        in1=s13.to_broadcast([128, NT, E]), op=AL.mult)
                nc.vector.tensor_tensor(out=w3, in0=e23,
                                        in1=s23.to_broadcast([128, NT, E]), op=AL.mult)
                nc.vector.tensor_add(wge[:], wge[:], t1e[:])
                al3 = al_sb[:].rearrange("p (o e) -> p o e", o=1)
                nc.vector.tensor_tensor(out=w3, in0=w3,
                                        in1=al3.to_broadcast([128, NT, E]), op=AL.mult)

                # msk = EQ1 + EQ2
                mske = rt.tile([128, NT * E], F32)
                nc.vector.tensor_add(mske[:], eq1[:], eq2[:])
                mk3 = mske[:].rearrange("p (i e) -> p i e", e=E)

                # inclusive cumsum along i (token-tile axis), all experts at once
                cumA = rt.tile([128, NT * E], F32)
                cumB = rt.tile([128, NT * E], F32)
                nc.vector.tensor_copy(out=cumA[:], in_=mske[:])
                src, dst = cumA, cumB
                for s in (1, 2, 4, 8, 16):
                    s3 = src[:].rearrange("p (i e) -> p i e", e=E)
                    d3 = dst[:].rearrange("p (i e) -> p i e", e=E)
                    nc.scalar.copy(out=d3[:, :s, :], in_=s3[:, :s, :])
                    nc.vector.tensor_tensor(out=d3[:, s:, :], in0=s3[:, s:, :],
                                            in1=s3[:, :NT - s, :], op=AL.add)
                    src, dst = dst, src
                incl = src
                ic3 = incl[:].rearrange("p (i e) -> p i e", e=E)

                rowtot = rt.tile([128, E], F32)
                nc.vector.tensor_copy(
                    out=rowtot[:].rearrange("p (o e) -> p o e", o=1),
                    in_=ic3[:, NT - 1:NT, :])
                pro = pst.tile([128, E], F32, space="PSUM", tag="pro")
                nc.tensor.matmul(pro[:], utri_sb[:], rowtot[:], start=True, stop=True)
                rowoff = rt.tile([128, E], F32)
                nc.vector.tensor_copy(out=rowoff[:], in_=pro[:])

                # slot_e(t) = rowoff_e + incl_e - msk_e   [128, NT, E]
                slte = rt.tile([128, NT * E], F32)
                sl3 = slte[:].rearrange("p (i e) -> p i e", e=E)
                nc.vector.tensor_sub(slte[:], incl[:], mske[:])
                ro3 = rowoff[:].rearrange("p (o e) -> p o e", o=1)
                nc.vector.tensor_tensor(out=sl3, in0=sl3,
                                        in1=ro3.to_broadcast([128, NT, E]), op=AL.add)

                # ---- own-expert selection for compaction ----
                msk = rt.tile([128, NT], F32)
                oh3 = oh_sb[:].rearrange("p (o e) -> p o e", o=1)
                sel_tmp = rt.tile([128, NT * E], F32)
                st3 = sel_tmp[:].rearrange("p (i e) -> p i e", e=E)
                nc.vector.tensor_tensor(out=st3, in0=mk3,
                                        in1=oh3.to_broadcast([128, NT, E]), op=AL.mult)
                nc.vector.tensor_reduce(out=msk[:], in_=st3,
                                        axis=mybir.AxisListType.X, op=AL.add)
                slot = rt.tile([128, NT], F32)
                nc.vector.tensor_tensor(out=st3, in0=sl3,
                                        in1=oh3.to_broadcast([128, NT, E]), op=AL.mult)
                nc.vector.tensor_reduce(out=slot[:], in_=st3,
                                        axis=mybir.AxisListType.X, op=AL.add)
                wgt = rt.tile([128, NT], F32)
                nc.vector.tensor_tensor(out=st3, in0=w3,
                                        in1=oh3.to_broadcast([128, NT, E]), op=AL.mult)
                nc.vector.tensor_reduce(out=wgt[:], in_=st3,
                                        axis=mybir.AxisListType.X, op=AL.add)

                # masked-out -> +1e6 so compaction misses every slot column
                nc.vector.scalar_tensor_tensor(
                    out=slot[:], in0=msk[:], scalar=-1e6, in1=slot[:],
                    op0=AL.mult, op1=AL.add)
                nc.vector.tensor_scalar_add(slot[:], slot[:], 1e6)

                # w split into exact bf16 hi/lo for the compaction matmul
                whi = rt.tile([128, NT], BF16)
                nc.vector.tensor_copy(out=whi[:], in_=wgt[:])
                whi32 = rt.tile([128, NT], F32)
                nc.vector.tensor_copy(out=whi32[:], in_=whi[:])
                wlo32 = rt.tile([128, NT], F32)
                nc.vector.tensor_sub(wlo32[:], wgt[:], whi32[:])

                # lhs5[p, i, :] = [p, i, 1, w_hi, w_lo]  (bf16)
                lhs5 = rt.tile([128, NT * 5], BF16)
                l53 = lhs5[:].rearrange("p (i c) -> p i c", c=5)
                nc.vector.tensor_copy(out=l53[:, :, 0:3],
                                      in_=io3_sb[:].rearrange("p (i c) -> p i c", c=3))
                nc.vector.tensor_copy(out=l53[:, :, 3:4],
                                      in_=whi32[:].rearrange("p (i o) -> p i o", o=1))
                nc.vector.tensor_copy(out=l53[:, :, 4:5],
                                      in_=wlo32[:].rearrange("p (i o) -> p i o", o=1))

                # compaction matmuls: rows = [sum p*EQ, sum i*EQ, colsum, w_hi, w_lo]
                slot16 = rt.tile([128, NT], mybir.dt.float16)
                nc.vector.tensor_copy(out=slot16[:], in_=slot[:])
                ccs = [(0, 512), (512, 512), (1024, C - 1024)]
                pqs = []
                for (_, n) in ccs:
                    pq_t = psq.tile([5, n], F32, space="PSUM", tag="pq")
                    pqs.append(pq_t)
                for i in range(NT):
                    eq = eqp.tile([128, C], BF16, tag="eqt")
                    nc.vector.tensor_tensor(
                        out=eq[:], in0=slot16[:, i:i + 1].to_broadcast([128, C]),
                        in1=srow_sb[:], op=AL.is_equal)
                    for ci, (c0, n) in enumerate(ccs):
                        nc.tensor.matmul(pqs[ci][:], lhs5[:, i * 5:(i + 1) * 5],
                                         eq[:, c0:c0 + n],
                                         start=(i == 0), stop=(i == NT - 1))

                # transpose [5, C] -> per-slot columns [128, NC9, 5]
                qs = rt.tile([5, C], F32)
                for ci, (c0, n) in enumerate(ccs):
                    nc.scalar.copy(out=qs[:, c0:c0 + n], in_=pqs[ci][:])
                ptc = pst.tile([128, NC9 * 5], F32, space="PSUM", tag="ptc")
                for t9 in range(NC9):
                    nc.tensor.transpose(ptc[:, t9 * 5:(t9 + 1) * 5],
                                        qs[:, t9 * 128:(t9 + 1) * 128], idf[:5, :5])
                qcols = rt.tile([128, NC9 * 5], F32)
                nc.scalar.copy(out=qcols[:], in_=ptc[:])
                q3 = qcols[:].rearrange("p (t c) -> p t c", c=5)

                gi_f = rt.tile([128, NC9], F32)
                g3 = gi_f[:].rearrange("p (t o) -> p t o", o=1)
                nc.vector.scalar_tensor_tensor(
                    out=g3, in0=q3[:, :, 1:2], scalar=128.0,
                    in1=q3[:, :, 0:1], op0=AL.mult, op1=AL.add)
                nc.vector.scalar_tensor_tensor(
                    out=g3, in0=q3[:, :, 2:3], scalar=-1e6,
                    in1=g3, op0=AL.mult, op1=AL.add)
                nc.vector.tensor_scalar_add(gi_f[:], gi_f[:], 1e6)
                nc.vector.tensor_copy(out=gcols[:], in_=gi_f[:])
                nc.vector.tensor_tensor(
                    out=wcols[:].rearrange("p (t o) -> p t o", o=1),
                    in0=q3[:, :, 3:4], in1=q3[:, :, 4:5], op=AL.add)

                # combine addresses: addr_e = slot_e + e*C; A1/A2 select top-1/2
                adre = rt.tile([128, NT * E], F32)
                ad3 = adre[:].rearrange("p (i e) -> p i e", e=E)
                ec3 = ec_sb[:].rearrange("p (o e) -> p o e", o=1)
                nc.vector.tensor_tensor(out=ad3, in0=sl3,
                                        in1=ec3.to_broadcast([128, NT, E]), op=AL.add)
                a12 = rt.tile([128, 2 * NT], F32)
                tmp3 = t1e  # reuse as scratch
                tm3 = tmp3[:].rearrange("p (i e) -> p i e", e=E)
                nc.vector.tensor_tensor(out=tm3, in0=ad3, in1=e13, op=AL.mult)
                nc.vector.tensor_reduce(
                    out=a12[:, :NT], in_=tm3,
                    axis=mybir.AxisListType.X, op=AL.add)
                nc.vector.tensor_tensor(out=tm3, in0=ad3, in1=e23, op=AL.mult)
                nc.vector.tensor_reduce(
                    out=a12[:, NT:], in_=tm3,
                    axis=mybir.AxisListType.X, op=AL.add)

                # select own-token columns: ownT = sel2.T @ (A12.T)  -> [8, 128]
                with tc.tile_pool(name="psel", bufs=1, space="PSUM") as psel:
                    pt12 = psel.tile([64, 128], F32, space="PSUM", tag="pt12")
                    nc.tensor.transpose(pt12[:], a12[:], idf[:])
                    a12T = rt.tile([64, 128], F32)
                    nc.vector.tensor_copy(out=a12T[:], in_=pt12[:])
                    pown = psel.tile([E, 128], F32, space="PSUM", tag="pown")
                    nc.tensor.matmul(pown[:], sel_sb[:], a12T[:], start=True, stop=True)
                    ownT = rt.tile([E, 128], F32)
                    nc.vector.tensor_copy(out=ownT[:], in_=pown[:])
                    pow2 = psel.tile([128, E], F32, space="PSUM", tag="pow2")
                    nc.tensor.transpose(pow2[:], ownT[:], idf[:E, :E])
                    ownf = rt.tile([128, E], F32)
                    nc.vector.tensor_copy(out=ownf[:], in_=pow2[:])
                    nc.vector.tensor_copy(out=a1own[:], in_=ownf[:, :NTO])
                    nc.vector.tensor_copy(out=a2own[:], in_=ownf[:, NTO:])

            # =================== phase 2: gather + expert MLP ===================
            with (
                tc.tile_pool(name="xth", bufs=1) as xthp,
                tc.tile_pool(name="hh", bufs=1) as hhp,
                tc.tile_pool(name="yac", bufs=1) as yacp,
                tc.tile_pool(name="w1p", bufs=2) as w1p,
                tc.tile_pool(name="w2p", bufs=3) as w2p,
                tc.tile_pool(name="stg", bufs=2) as stg,
                tc.tile_pool(name="ocv", bufs=2) as ocv,
            ):
                xth = xthp.tile([128, NHC * C], BF16)
                xt3 = xth[:].rearrange("p (h c) -> p h c", c=C)
                hh = hhp.tile([128, LFC * C], BF16)
                hh3 = hh[:].rearrange("p (f c) -> p f c", c=C)
                yacc = yacp.tile([128, NC9 * H], BF16)
                ya3 = yacc[:].rearrange("p (t h) -> p t h", h=H)

                # gather + transpose X^T for all capacity slots: [128, NHC, C].
                # One xbar DMA transpose per token tile: [128 tok, 2048 h] ->
                # [128, 16 hc, 128 tok] with h = hc*128 + p.
                with tc.tile_pool(name="xgp", bufs=2) as xgp:
                    for t9 in range(NC9):
                        xg = xgp.tile([128, H], BF16, tag="xg")
                        nc.gpsimd.indirect_dma_start(
                            out=xg[:], out_offset=None, in_=hx[:, :],
                            in_offset=IndirectOffsetOnAxis(
                                ap=gcols[:, t9:t9 + 1], axis=0),
                            bounds_check=T - 1, oob_is_err=False)
                        nc.sync.dma_start_transpose(
                            out=xt3[:, :, t9 * 128:(t9 + 1) * 128], in_=xg[:])

                with (
                    tc.tile_pool(name="psf", bufs=2, space="PSUM") as psf,
                    tc.tile_pool(name="psy", bufs=2, space="PSUM") as psy,
                ):
                  for fq in range(NFQ):
                    # fc1 quarter: hh = gelu(W1q.T @ X^T + b1q), full C.
                    # hc is the outer loop so each weight tile stays stationary
                    # for the 3 token chunks (one LDWEIGHTS per 3 matmuls).
                    for fcp in range(LFC // 2):
                        w1t = w1p.tile([128, 2 * NHC * 128], BF16, tag="w1t")
                        nc.sync.dma_start(w1t[:], w1[fq * (LFC // 2) + fcp, :, :])
                        for j in range(2):
                            lfc = fcp * 2 + j
                            fc = fq * LFC + lfc
                            pfs = [psf.tile([128, CC3], F32, space="PSUM",
                                            tag=f"pf{k}", name=f"pf{k}")
                                   for k in range(3)]
                            for hc in range(NHC):
                                for tc3 in range(3):
                                    nc.tensor.matmul(
                                        pfs[tc3][:],
                                        w1t[:, (j * NHC + hc) * 128:(j * NHC + hc + 1) * 128],
                                        xt3[:, hc, tc3 * CC3:(tc3 + 1) * CC3],
                                        start=(hc == 0), stop=(hc == NHC - 1))
                            for tc3 in range(3):
                                nc.scalar.activation(
                                    hh3[:, lfc, tc3 * CC3:(tc3 + 1) * CC3], pfs[tc3][:],
                                    AF.Gelu, bias=b1_sb[:, fc:fc + 1])

                    # fc2 quarter: accumulate into yacc
                    for h4 in range(NH4):
                        w2t = w2p.tile([128, LFC * H4], BF16, tag="w2t")
                        nc.sync.dma_start(w2t[:], w2[fq, h4, :, :])
                        w23 = w2t[:].rearrange("p (f h) -> p f h", h=H4)
                        for tt in range(NC9):
                            py = psy.tile([128, H4], F32, space="PSUM", tag="py")
                            for lfc in range(LFC):
                                nc.tensor.matmul(
                                    py[:], hh3[:, lfc, tt * 128:(tt + 1) * 128],
                                    w23[:, lfc, :],
                                    start=(lfc == 0), stop=(lfc == LFC - 1))
                            yv = ya3[:, tt, h4 * H4:(h4 + 1) * H4]
                            if fq == 0:
                                nc.vector.tensor_add(yv, py[:],
                                                     b2_sb[:, h4 * H4:(h4 + 1) * H4])
                            elif fq < NFQ - 1:
                                nc.vector.tensor_add(yv, yv, py[:])
                            else:
                                yfin = stg.tile([128, H4], F32, tag="yfin")
                                nc.vector.tensor_add(yfin[:], yv, py[:])
                                ystb = stg.tile([128, H4], BF16, tag="ystb")
                                nc.vector.tensor_scalar(
                                    out=ystb[:], in0=yfin[:],
                                    scalar1=wcols[:, tt:tt + 1],
                                    scalar2=None, op0=AL.mult)
                                nc.sync.dma_start(
                                    yexps[h4 // 2][tt * 128:(tt + 1) * 128,
                                                   (h4 % 2) * H4:(h4 % 2 + 1) * H4],
                                    ystb[:])
                        if fq == NFQ - 1 and h4 == 1:
                            # first column half is complete: AllGather it now so
                            # the collective overlaps the remaining fc2 work.
                            nc.gpsimd.collective_compute(
                                "AllGather", AL.bypass,
                                replica_groups=[list(range(E))],
                                ins=[yexps[0].opt()], outs=[yalls[0].opt()],
                                unique_tensors="Yes")

                # ---- combine: gather the two contribution rows per own token.
                # The first half's gathers are queued before the second AG so
                # they run while the second half's collective is in flight.
                for hp in range(2):
                    for tt in range(NTO):
                        ya = ocv.tile([128, 2 * H4], F32, tag="ya")
                        nc.gpsimd.indirect_dma_start(
                            out=ya[:], out_offset=None, in_=yalls[hp][:, :],
                            in_offset=IndirectOffsetOnAxis(
                                ap=a1own[:, tt:tt + 1], axis=0),
                            bounds_check=E * C - 1, oob_is_err=False)
                        nc.gpsimd.indirect_dma_start(
                            out=ya[:], out_offset=None, in_=yalls[hp][:, :],
                            in_offset=IndirectOffsetOnAxis(
                                ap=a2own[:, tt:tt + 1], axis=0),
                            bounds_check=E * C - 1, oob_is_err=False,
                            compute_op=AL.add)
                        nc.sync.dma_start(
                            out[tt * 128:(tt + 1) * 128,
                                hp * 2 * H4:(hp + 1) * 2 * H4],
                            ya[:])
                    if hp == 0:
                        nc.gpsimd.collective_compute(
                            "AllGather", AL.bypass,
                            replica_groups=[list(range(E))],
                            ins=[yexps[1].opt()], outs=[yalls[1].opt()],
                            unique_tensors="Yes")

    nc.compile()
    return nc


def _host_prep(inputs):
    x = np.ascontiguousarray(inputs["hidden_states"].reshape(T, H).astype(np.float32))
    ht = np.ascontiguousarray(x.T)
    hx = x.astype(BF16NP)
    gw = np.ascontiguousarray(inputs["gate_w"].astype(np.float32))
    gb = np.ascontiguousarray(inputs["gate_b"].astype(np.float32).reshape(E, 1))
    srow = np.ascontiguousarray(
        np.broadcast_to(np.arange(C, dtype=np.float16), (128, C)))
    utri = np.triu(np.ones((128, 128), np.float32), k=1)
    io3 = np.empty((128, NT, 3), BF16NP)
    io3[:, :, 0] = np.arange(128, dtype=np.float32)[:, None]
    io3[:, :, 1] = np.arange(NT, dtype=np.float32)[None, :]
    io3[:, :, 2] = 1.0
    alb = np.ascontiguousarray(
        np.broadcast_to(inputs["alpha"].astype(np.float32), (128, E)))
    ecol = np.ascontiguousarray(
        np.broadcast_to((np.arange(E) * C).astype(np.float32), (128, E)))

    maps = []
    for e in range(E):
        w1e = inputs["fc1_w"][e].astype(BF16NP)          # [H, F]
        w1p = np.ascontiguousarray(
            w1e.reshape(NHC, 128, NFC // 2, 2, 128).transpose(2, 1, 3, 0, 4)
        ).reshape(NFC // 2, 128, 2 * NHC * 128)
        w2e = inputs["fc2_w"][e].astype(BF16NP)          # [F, H]
        w2p = np.ascontiguousarray(
            w2e.reshape(NFQ, LFC, 128, NH4, H4).transpose(0, 3, 2, 1, 4)
        ).reshape(NFQ, NH4, 128, LFC * H4)
        b1e = np.ascontiguousarray(
            inputs["fc1_b"][e].astype(np.float32).reshape(NFC, 128).T)
        b2e = np.ascontiguousarray(
            np.broadcast_to(inputs["fc2_b"][e].astype(BF16NP), (128, H)))
        ohe = np.zeros((128, E), np.float32)
        ohe[:, e] = 1.0
        hts = np.ascontiguousarray(ht[:, e * TS:(e + 1) * TS])
        sel = np.zeros((64, E), np.float32)
        for j in range(NTO):
            sel[e * NTO + j, j] = 1.0
            sel[NT + e * NTO + j, NTO + j] = 1.0
        maps.append({
            "hx": hx, "hts": hts, "gw": gw, "gb": gb,
            "w1": w1p, "b1": b1e, "w2": w2p, "b2": b2e,
            "alb": alb, "ohb": ohe, "ecol": ecol, "sel2": sel,
            "io3": io3, "srow": srow, "utri": utri,
        })
    return maps


def kernel(**inputs):
    import os
    if "nc" not in _CACHE:
        _CACHE["nc"] = _build()
    nc = _CACHE["nc"]
    maps = _host_prep(inputs)
    kw = {}
    if os.environ.get("MOE_TRACE"):
        kw["trace"] = True
        td = os.environ.get("MOE_TRACE_DIR")
        if td:
            os.makedirs(td, exist_ok=True)
            kw["tmpdir"] = td
    bres = run_bass_kernel_spmd(nc, maps, list(range(E)), **kw)
    _CACHE["last"] = bres
    res = bres.results
    outp = np.concatenate([res[e]["out"] for e in range(E)], axis=0)
    return outp.reshape(inputs["hidden_states"].shape).astype(np.float32)


if __name__ == "__main__":
    data = np.load("/root/problem/work/inputs.npz")
    out = kernel(**{k: data[k] for k in data.files})
    print("kernel output:", out.shape, out.dtype)
